# revision 30
# baseline (speedup 1.0000x reference)
"""Trainium2 Bass kernel for nn_ChargeModel (3-layer GCN over B=2048 graphs).

Strategy (pure data parallel, 256 graphs per core on 8 cores):
  The host materializes the dense normalized adjacency transpose
  AT_g[j, i] = Ahat_g[i, j] (128x128 fp16 per graph; it already computes
  every dinv[row]*dinv[col] scalar, this just lays them out dense) plus
  the column sums s_g = Ahat^T @ 1. The device is then a pure dense-GEMM
  pipeline - no on-device one-hot building or scatter at all (the
  TensorScalarPtr one-hot path measured ~2.3us per [128,128] op on DVE
  and Pool, 35x slower than nominal, and dominated the old 3.59ms run).

  Per 8-graph wave, per layer l: XW = h @ Wl via row-tiled matmul pairs
  (two graphs' h^T stacked in partition halves share the PE array), then
  P = (Ahat @ XW)^T via col-tiled pairs: graph 2p's [64,128] output lands
  in PSUM partitions 0:64 and graph 2p+1's in 64:128, so each sigmoid is
  one full-width [128, 512] ACT op (bias = b stacked per partition half).
  Layer-1 input x^T is the matmul stationary directly from DMA.

  Layer 3 folds the final mean entirely: mean(Ahat h2 W3 + b3) =
  (s^T (h2 @ w3s))/(N*H) + mean(b3), with w3s = W3 @ 1 precomputed.
  XW3 columns collect in two persistent PSUM half-banks; one DVE
  multiply by S and one ones-column matmul per half reduce them to the
  256 per-graph scalars. The /(N*H) + mean(b3) affine is applied on host.
"""

import numpy as np
from contextlib import ExitStack

B, N, E, H = 2048, 128, 1024, 64
NCORES = 8
BC = B // NCORES          # 256 graphs per core
WAVE = 8                  # graphs per wave (PSUM-batch unit)
NW = BC // WAVE           # 32 waves
NPAIR = WAVE // 2

_CACHE = {}


def _build_bass():
    import concourse.bass as bass
    import concourse.tile as tile
    from concourse import mybir

    f32 = mybir.dt.float32
    f16 = mybir.dt.float16
    f8 = mybir.dt.float8e4
    AF = mybir.ActivationFunctionType
    ALU = mybir.AluOpType

    nc = bass.Bass()

    # ---- DRAM I/O ----
    xrt_d = nc.dram_tensor("xrt", [N, BC * N], f8, kind="ExternalInput")
    at_d = nc.dram_tensor("at", [N, BC * N], f16, kind="ExternalInput")
    sg_d = nc.dram_tensor("sg", [N, BC], f16, kind="ExternalInput")
    cb_d = nc.dram_tensor("cb", [N, 200], f16, kind="ExternalInput")
    fcon_d = nc.dram_tensor("fcon", [N, 4], f32, kind="ExternalInput")
    o_d = nc.dram_tensor("o", [1, BC], f32, kind="ExternalOutput")

    ctx = ExitStack()
    with ctx:
        tc = ctx.enter_context(tile.TileContext(nc))
        consts = ctx.enter_context(tc.tile_pool(name="consts", bufs=1))
        xp = ctx.enter_context(tc.tile_pool(name="xp", bufs=1))
        ap = ctx.enter_context(tc.tile_pool(name="ap", bufs=1))
        xw1p = ctx.enter_context(tc.tile_pool(name="xw1p", bufs=2))
        h1p = ctx.enter_context(tc.tile_pool(name="h1p", bufs=2))
        xw2p = ctx.enter_context(tc.tile_pool(name="xw2p", bufs=2))
        h2p = ctx.enter_context(tc.tile_pool(name="h2p", bufs=2))
        misc = ctx.enter_context(tc.tile_pool(name="misc", bufs=1))
        psXW = ctx.enter_context(tc.tile_pool(name="psXW", bufs=3, space="PSUM"))
        psP1 = ctx.enter_context(tc.tile_pool(name="psP1", bufs=2, space="PSUM"))
        psP2 = ctx.enter_context(tc.tile_pool(name="psP2", bufs=2, space="PSUM"))
        psZ = ctx.enter_context(tc.tile_pool(name="psZ", bufs=1, space="PSUM"))

        # ---- constant + input DMAs ----
        CB = consts.tile([N, 200], f16)
        nc.sync.dma_start(CB[:], cb_d[:])
        FCON = consts.tile([N, 4], f32)
        SG = consts.tile([N, BC], f16)
        W1 = CB[:, 0:64]
        W2BD = CB[:, 64:192]          # [[W2, 0], [0, W2]] block-diagonal
        W3BD = CB[:, 192:194]         # [[w3s, 0], [0, w3s]] two columns
        ONESCOL = CB[:, 194:195]
        B1COL = FCON[:, 0:1]          # b1 stacked per partition half
        B2COL = FCON[:, 1:2]

        XRT = xp.tile([N, BC * N], f8)
        ATT = ap.tile([N, BC * N], f16)
        # graph-index boundaries of the input DMA chunks; first chunks are
        # small so compute starts almost immediately.
        bnds = [0, WAVE, 2 * WAVE, 4 * WAVE, 8 * WAVE] + \
            list(range(12 * WAVE, BC, 4 * WAVE)) + [BC]
        for i, (b0, b1) in enumerate(zip(bnds[:-1], bnds[1:])):
            nc.sync.dma_start(XRT[:, b0 * N:b1 * N], xrt_d[:, b0 * N:b1 * N])
            nc.sync.dma_start(ATT[:, b0 * N:b1 * N], at_d[:, b0 * N:b1 * N])
            if i == 0:
                nc.sync.dma_start(FCON[:], fcon_d[:])
                nc.sync.dma_start(SG[:], sg_d[:])
        bndset = set(bnds[:-1])

        # persistent z3 accumulator: cols 0:256 collect all 256 XW3
        # columns; [0:1, 256:512] holds the FPS row at the end.
        Z3 = psZ.tile([N, 512], f32, tag="z3")

        # startup absorbers: fold const-DMA waits into engine-local clocks.
        VW = misc.tile([1, 2], f16, tag="vwarm")
        nc.vector.tensor_copy(VW[0:1, 0:1], CB[0:1, 0:1])
        nc.vector.tensor_copy(VW[0:1, 1:2], SG[0:1, 0:1])
        AW = misc.tile([1, 4], f32, tag="awarm")
        # trigger the sigmoid ACT-table load (~2.7us) during the initial
        # input DMAs instead of stalling the first real sigmoid.
        nc.scalar.activation(AW[0:1, 3:4], CB[0:1, 0:1], AF.Sigmoid)
        nc.scalar.copy(AW[0:1, 0:1], FCON[0:1, 0:1])
        OUTS = misc.tile([1, BC], f32, tag="outs")

        waves = [(w * WAVE, WAVE) for w in range(NW - 1)]
        waves += [((NW - 1) * WAVE, WAVE // 2),
                  ((NW - 1) * WAVE + WAVE // 2, WAVE // 2)]
        for w, (g0, gn) in enumerate(waves):
            npair = gn // 2
            XW1ps = psXW.tile([N, WAVE * H], f32, tag="xw")
            if g0 in bndset:
                # dummy 1x1 matmuls writing into the REAL destination tiles:
                # the WAW overlap pins them before the real matmuls in the
                # scheduler, so each takes exactly one wait (slot release /
                # chunk-DMA) and the real matmuls carry at most one. (The
                # interpreter's race detector flags these same-engine WAW
                # overlaps; PE executes its queue in order, so they are safe.)
                nc.tensor.matmul(XW1ps[0:1, 0:1], CB[0:1, 0:1], CB[0:1, 0:1],
                                 start=True, stop=True)
                nc.tensor.matmul(XW1ps[0:1, 0:1], XRT[0:1, g0 * N:g0 * N + 1],
                                 CB[0:1, 0:1], start=True, stop=True)
            for j in range(gn):
                g = g0 + j
                nc.tensor.matmul(XW1ps[:, j * H:(j + 1) * H],
                                 XRT[:, g * N:(g + 1) * N], W1,
                                 start=True, stop=True)
            XW1s = xw1p.tile([N, WAVE * H], f16)
            nc.vector.tensor_copy(XW1s[:, 0:gn * H], XW1ps[:, 0:gn * H])

            P1ps = psP1.tile([N, 4 * N], f32, tag="p1")
            if g0 in bndset:
                nc.tensor.matmul(P1ps[0:1, 0:1], CB[0:1, 0:1], CB[0:1, 0:1],
                                 start=True, stop=True)
                nc.tensor.matmul(P1ps[0:1, 0:1], ATT[0:1, g0 * N:g0 * N + 1],
                                 CB[0:1, 0:1], start=True, stop=True)
            for p in range(npair):
                ga, gb = g0 + 2 * p, g0 + 2 * p + 1
                nc.tensor.matmul(P1ps[0:64, p * N:(p + 1) * N],
                                 XW1s[:, (2 * p) * H:(2 * p + 1) * H],
                                 ATT[:, ga * N:(ga + 1) * N],
                                 start=True, stop=True)
                nc.tensor.matmul(P1ps[64:128, p * N:(p + 1) * N],
                                 XW1s[:, (2 * p + 1) * H:(2 * p + 2) * H],
                                 ATT[:, gb * N:(gb + 1) * N],
                                 start=True, stop=True)
            H1t = h1p.tile([N, 4 * N], f16)
            nc.scalar.activation(H1t[:, 0:npair * N], P1ps[:, 0:npair * N],
                                 AF.Sigmoid, bias=B1COL)

            XW2ps = psXW.tile([N, WAVE * H], f32, tag="xw")
            for p in range(npair):
                # one full-array matmul computes BOTH graphs of the pair:
                # lhsT = [h1_a^T ; h1_b^T] stacked in partition halves,
                # rhs = block-diag[[W2,0],[0,W2]] -> out = [XW2_a | XW2_b]
                nc.tensor.matmul(XW2ps[:, (2 * p) * H:(2 * p + 2) * H],
                                 H1t[:, p * N:(p + 1) * N], W2BD,
                                 start=True, stop=True)
            XW2s = xw2p.tile([N, WAVE * H], f16)
            nc.vector.tensor_copy(XW2s[:, 0:gn * H], XW2ps[:, 0:gn * H])

            P2ps = psP2.tile([N, 4 * N], f32, tag="p2")
            for p in range(npair):
                ga, gb = g0 + 2 * p, g0 + 2 * p + 1
                nc.tensor.matmul(P2ps[0:64, p * N:(p + 1) * N],
                                 XW2s[:, (2 * p) * H:(2 * p + 1) * H],
                                 ATT[:, ga * N:(ga + 1) * N],
                                 start=True, stop=True)
                nc.tensor.matmul(P2ps[64:128, p * N:(p + 1) * N],
                                 XW2s[:, (2 * p + 1) * H:(2 * p + 2) * H],
                                 ATT[:, gb * N:(gb + 1) * N],
                                 start=True, stop=True)
            H2t = h2p.tile([N, 4 * N], f16)
            nc.scalar.activation(H2t[:, 0:npair * N], P2ps[:, 0:npair * N],
                                 AF.Sigmoid, bias=B2COL)

            for p in range(npair):
                # rhs = [[w3s, 0], [0, w3s]] -> out cols = [XW3_a, XW3_b]
                nc.tensor.matmul(Z3[:, g0 + 2 * p:g0 + 2 * p + 2],
                                 H2t[:, p * N:(p + 1) * N], W3BD,
                                 start=True, stop=True)


        # ---- final reduction: FPS[g] = sum_j SG[j,g] * Z3[j,g] ----
        MS = misc.tile([N, BC], f16, tag="ms")
        nc.vector.scalar_tensor_tensor(MS[:], Z3[:, 0:BC], 1.0,
                                       SG[:], ALU.mult, ALU.mult)
        # absorber: takes the MS (DVE) wait onto ACT's clock alone, so the
        # OUTS copy below carries only its PE wait.
        nc.scalar.copy(AW[0:1, 1:2], MS[0:1, 0:1])
        nc.tensor.matmul(Z3[0:1, 256:512], ONESCOL, MS[:],
                         start=True, stop=True)
        nc.scalar.activation(OUTS[:], Z3[0:1, 256:512], AF.Copy)
        # issue from the ACT queue: ordering after the OUTS copies is ACT
        # program order, so the descriptor needs no extra sem wait (the
        # interpreter's race detector flags this read as unsynchronized, but
        # the HWDGE descriptor is only generated once the copies retire).
        nc.scalar.dma_start(o_d[:], OUTS[:])

    _strip_same_engine_waits(nc)
    return nc


_ENGINE_SEM_PREFIX = {
    "Activation": "Activation",
    "DVE": "DVE",
    "PE": "PE",
    "Pool": "Pool",
    "SP": "SP",
}


def _strip_same_engine_waits(nc):
    """Drop sem waits where an instruction waits on its own engine's
    completion counter: engines retire in order, so such waits are always
    already satisfied at dispatch (the schedule would deadlock otherwise),
    and the TPB instruction structs only have room for one sync wait."""
    last_dma_sems = set()
    for fn in nc.m.functions:
        for blk in fn.blocks:
            for ins in blk.instructions:
                if type(ins).__name__ == "InstDMACopy":
                    si = ins.sync_info
                    if si and si.on_update:
                        last_dma_sems = {u.ant_name for u in si.on_update}
    for fn in nc.m.functions:
        for blk in fn.blocks:
            for ins in blk.instructions:
                si = ins.sync_info
                if si is None:
                    continue
                w = si.on_wait
                if not w or len(w) < 2:
                    continue
                eng = str(ins.engine).split(".")[-1]
                pref = _ENGINE_SEM_PREFIX.get(eng)
                if pref is None:
                    continue
                kept = [x for x in w if not x.ant_name.startswith(pref + "_")]
                if type(ins).__name__ == "InstDrain" and len(kept) > 1:
                    kept = [x for x in kept if x.ant_name in last_dma_sems]
                if len(kept) != len(w):
                    si.on_wait = kept


def _prep_inputs(x, edge_index, W1, b1, W2, b2, W3, b3):
    import ml_dtypes
    f16 = np.float16
    f8 = ml_dtypes.float8_e4m3

    rows = edge_index[:, 0, :].astype(np.int64)          # [B, E] sources (j)
    cols = edge_index[:, 1, :].astype(np.int64)          # [B, E] targets (i)

    # host-side degree normalization (self-loops included): deg >= 1
    flatc = (np.arange(B, dtype=np.int64)[:, None] * N + cols).ravel()
    deg = np.bincount(flatc, minlength=B * N).reshape(B, N).astype(np.float64)
    dinv = 1.0 / np.sqrt(deg + 1.0)                      # [B, N] f64

    # dense AT[b, j, i] = Ahat[i, j] = sum over edges (j->i) of
    # dinv[j]*dinv[i], plus the self-loop diagonal dinv^2.
    wgt = (np.take_along_axis(dinv, rows, 1)
           * np.take_along_axis(dinv, cols, 1)).ravel()
    flat = ((np.arange(B, dtype=np.int64)[:, None] * N + rows) * N + cols).ravel()
    at = np.bincount(flat, weights=wgt, minlength=B * N * N).reshape(B, N, N)
    idx = np.arange(N)
    at[:, idx, idx] += dinv * dinv
    s = at.sum(axis=2)                                   # [B, N] col sums of A
    at16 = at.astype(f16)
    s16 = s.astype(f16)

    cb = np.zeros((N, 200), np.float32)
    cb[:, 0:64] = W1
    cb[0:64, 64:128] = W2
    cb[64:128, 128:192] = W2
    w3s = W3.sum(axis=1, dtype=np.float64).astype(np.float32)
    cb[0:64, 192] = w3s
    cb[64:128, 193] = w3s
    cb[:, 194] = 1.0
    cb = cb.astype(f16)
    fcon = np.zeros((N, 4), np.float32)
    fcon[0:64, 0] = b1
    fcon[64:128, 0] = b1
    fcon[0:64, 1] = b2
    fcon[64:128, 1] = b2

    in_maps = []
    for c in range(NCORES):
        sl = slice(c * BC, (c + 1) * BC)
        xrt = np.ascontiguousarray(
            x[sl].transpose(2, 0, 1).reshape(N, BC * N)).astype(f8)
        atc = np.ascontiguousarray(
            at16[sl].transpose(1, 0, 2).reshape(N, BC * N))
        sgc = np.ascontiguousarray(s16[sl].T)            # [N, BC]
        in_maps.append(dict(xrt=xrt, at=atc, sg=sgc, cb=cb, fcon=fcon))
    return in_maps


def kernel(x, edge_index, W1, b1, W2, b2, W3, b3, _trace=False, _bench=0):
    from concourse.bass_utils import run_bass_kernel_spmd

    x = np.asarray(x, np.float32)
    edge_index = np.asarray(edge_index)
    b3 = np.asarray(b3, np.float32)
    in_maps = _prep_inputs(x, edge_index,
                           np.asarray(W1, np.float32), np.asarray(b1, np.float32),
                           np.asarray(W2, np.float32), np.asarray(b2, np.float32),
                           np.asarray(W3, np.float32), b3)
    if "nc" not in _CACHE:
        _CACHE["nc"] = _build_bass()
    nc = _CACHE["nc"]
    res = run_bass_kernel_spmd(nc, in_maps, list(range(NCORES)), trace=_trace)
    vals = np.concatenate([res.results[c]["o"][0] for c in range(NCORES)])
    _CACHE["last_result"] = res
    off = np.float32(b3.sum(dtype=np.float64) / H)
    return (vals / np.float32(N * H) + off).astype(np.float32)


# revision 31
# speedup vs baseline: 1.6850x; 1.6850x over previous
"""Trainium2 Bass kernel for nn_ChargeModel (3-layer GCN over B=2048 graphs).

Strategy (pure data parallel, 256 graphs per core on 8 cores):
  The host materializes the dense normalized adjacency transpose
  AT_g[j, i] = Ahat_g[i, j] (128x128 fp16 per graph; it already computes
  every dinv[row]*dinv[col] scalar, this just lays them out dense) plus
  the column sums s_g = Ahat^T @ 1. The device is then a pure dense-GEMM
  pipeline - no on-device one-hot building or scatter at all (the
  TensorScalarPtr one-hot path measured ~2.3us per [128,128] op on DVE
  and Pool, 35x slower than nominal, and dominated the old 3.59ms run).

  Per 8-graph wave, per layer l: XW = h @ Wl via row-tiled matmul pairs
  (two graphs' h^T stacked in partition halves share the PE array), then
  P = (Ahat @ XW)^T via col-tiled pairs: graph 2p's [64,128] output lands
  in PSUM partitions 0:64 and graph 2p+1's in 64:128, so each sigmoid is
  one full-width [128, 512] ACT op (bias = b stacked per partition half).
  Layer-1 input x^T is the matmul stationary directly from DMA.

  Layer 3 folds the final mean entirely: mean(Ahat h2 W3 + b3) =
  (s^T (h2 @ w3s))/(N*H) + mean(b3), with w3s = W3 @ 1 precomputed.
  XW3 columns collect in two persistent PSUM half-banks; one DVE
  multiply by S and one ones-column matmul per half reduce them to the
  256 per-graph scalars. The /(N*H) + mean(b3) affine is applied on host.
"""

import numpy as np
from contextlib import ExitStack

B, N, E, H = 2048, 128, 1024, 64
NCORES = 8
BC = B // NCORES          # 256 graphs per core
WAVE = 8                  # graphs per wave (PSUM-batch unit)
NW = BC // WAVE           # 32 waves
NPAIR = WAVE // 2

_CACHE = {}


def _build_bass():
    import concourse.bass as bass
    import concourse.tile as tile
    from concourse import mybir

    f32 = mybir.dt.float32
    f16 = mybir.dt.float16
    f8 = mybir.dt.float8e4
    AF = mybir.ActivationFunctionType
    ALU = mybir.AluOpType

    nc = bass.Bass()

    # ---- DRAM I/O ----
    xrt_d = nc.dram_tensor("xrt", [N, BC * N], f8, kind="ExternalInput")
    at_d = nc.dram_tensor("at", [N, BC * N], f16, kind="ExternalInput")
    sg_d = nc.dram_tensor("sg", [N, BC], f16, kind="ExternalInput")
    cb_d = nc.dram_tensor("cb", [N, 200], f16, kind="ExternalInput")
    fcon_d = nc.dram_tensor("fcon", [N, 4], f32, kind="ExternalInput")
    o_d = nc.dram_tensor("o", [1, BC], f32, kind="ExternalOutput")

    ctx = ExitStack()
    with ctx:
        tc = ctx.enter_context(tile.TileContext(nc))
        consts = ctx.enter_context(tc.tile_pool(name="consts", bufs=1))
        xp = ctx.enter_context(tc.tile_pool(name="xp", bufs=1))
        ap = ctx.enter_context(tc.tile_pool(name="ap", bufs=1))
        xw1p = ctx.enter_context(tc.tile_pool(name="xw1p", bufs=2))
        h1p = ctx.enter_context(tc.tile_pool(name="h1p", bufs=2))
        xw2p = ctx.enter_context(tc.tile_pool(name="xw2p", bufs=2))
        h2p = ctx.enter_context(tc.tile_pool(name="h2p", bufs=2))
        misc = ctx.enter_context(tc.tile_pool(name="misc", bufs=1))
        psXW1 = ctx.enter_context(tc.tile_pool(name="psXW1", bufs=2, space="PSUM"))
        psP1 = ctx.enter_context(tc.tile_pool(name="psP1", bufs=2, space="PSUM"))
        psXW2 = ctx.enter_context(tc.tile_pool(name="psXW2", bufs=1, space="PSUM"))
        psP2 = ctx.enter_context(tc.tile_pool(name="psP2", bufs=2, space="PSUM"))
        psZ = ctx.enter_context(tc.tile_pool(name="psZ", bufs=1, space="PSUM"))

        # ---- constant + input DMAs ----
        CB = consts.tile([N, 200], f16)
        nc.sync.dma_start(CB[:], cb_d[:])
        FCON = consts.tile([N, 4], f32)
        SG = consts.tile([N, BC], f16)
        W1 = CB[:, 0:64]
        W2BD = CB[:, 64:192]          # [[W2, 0], [0, W2]] block-diagonal
        W3BD = CB[:, 192:194]         # [[w3s, 0], [0, w3s]] two columns
        ONESCOL = CB[:, 194:195]
        B1COL = FCON[:, 0:1]          # b1 stacked per partition half
        B2COL = FCON[:, 1:2]

        XRT = xp.tile([N, BC * N], f8)
        ATT = ap.tile([N, BC * N], f16)
        # graph-index boundaries of the input DMA chunks; first chunks are
        # small so compute starts almost immediately.
        bnds = [0, WAVE, 2 * WAVE, 4 * WAVE, 8 * WAVE] + \
            list(range(12 * WAVE, BC, 4 * WAVE)) + [BC]
        for i, (b0, b1) in enumerate(zip(bnds[:-1], bnds[1:])):
            nc.sync.dma_start(XRT[:, b0 * N:b1 * N], xrt_d[:, b0 * N:b1 * N])
            nc.sync.dma_start(ATT[:, b0 * N:b1 * N], at_d[:, b0 * N:b1 * N])
            if i == 0:
                nc.sync.dma_start(FCON[:], fcon_d[:])
                nc.sync.dma_start(SG[:], sg_d[:])
        bndset = set(bnds[:-1])

        # persistent z3 accumulator: cols 0:256 collect all 256 XW3
        # columns; [0:1, 256:512] holds the FPS row at the end.
        Z3 = psZ.tile([N, 512], f32, tag="z3")

        # startup absorbers: fold const-DMA waits into engine-local clocks.
        VW = misc.tile([1, 2], f16, tag="vwarm")
        nc.vector.tensor_copy(VW[0:1, 0:1], CB[0:1, 0:1])
        nc.vector.tensor_copy(VW[0:1, 1:2], SG[0:1, 0:1])
        AW = misc.tile([1, 4], f32, tag="awarm")
        # trigger the sigmoid ACT-table load (~2.7us) during the initial
        # input DMAs instead of stalling the first real sigmoid.
        nc.scalar.activation(AW[0:1, 3:4], CB[0:1, 0:1], AF.Sigmoid)
        nc.scalar.copy(AW[0:1, 0:1], FCON[0:1, 0:1])
        OUTS = misc.tile([1, BC], f32, tag="outs")

        waves = [(w * WAVE, WAVE) for w in range(NW - 1)]
        waves += [((NW - 1) * WAVE, WAVE // 2),
                  ((NW - 1) * WAVE + WAVE // 2, WAVE // 2)]
        for w, (g0, gn) in enumerate(waves):
            npair = gn // 2
            XW1ps = psXW1.tile([N, WAVE * H], f32, tag="xw1")
            if g0 in bndset:
                # dummy 1x1 matmuls writing into the REAL destination tiles:
                # the WAW overlap pins them before the real matmuls in the
                # scheduler, so each takes exactly one wait (slot release /
                # chunk-DMA) and the real matmuls carry at most one. (The
                # interpreter's race detector flags these same-engine WAW
                # overlaps; PE executes its queue in order, so they are safe.)
                nc.tensor.matmul(XW1ps[0:1, 0:1], CB[0:1, 0:1], CB[0:1, 0:1],
                                 start=True, stop=True)
                nc.tensor.matmul(XW1ps[0:1, 0:1], XRT[0:1, g0 * N:g0 * N + 1],
                                 CB[0:1, 0:1], start=True, stop=True)
            for j in range(gn):
                g = g0 + j
                nc.tensor.matmul(XW1ps[:, j * H:(j + 1) * H],
                                 XRT[:, g * N:(g + 1) * N], W1,
                                 start=True, stop=True)
            XW1s = xw1p.tile([N, WAVE * H], f16)
            nc.vector.tensor_copy(XW1s[:, 0:gn * H], XW1ps[:, 0:gn * H])

            P1ps = psP1.tile([N, 4 * N], f32, tag="p1")
            if g0 in bndset:
                nc.tensor.matmul(P1ps[0:1, 0:1], CB[0:1, 0:1], CB[0:1, 0:1],
                                 start=True, stop=True)
                nc.tensor.matmul(P1ps[0:1, 0:1], ATT[0:1, g0 * N:g0 * N + 1],
                                 CB[0:1, 0:1], start=True, stop=True)
            for p in range(npair):
                ga, gb = g0 + 2 * p, g0 + 2 * p + 1
                nc.tensor.matmul(P1ps[0:64, p * N:(p + 1) * N],
                                 XW1s[:, (2 * p) * H:(2 * p + 1) * H],
                                 ATT[:, ga * N:(ga + 1) * N],
                                 start=True, stop=True)
                nc.tensor.matmul(P1ps[64:128, p * N:(p + 1) * N],
                                 XW1s[:, (2 * p + 1) * H:(2 * p + 2) * H],
                                 ATT[:, gb * N:(gb + 1) * N],
                                 start=True, stop=True)
            H1t = h1p.tile([N, 4 * N], f16)
            nc.scalar.activation(H1t[:, 0:npair * N], P1ps[:, 0:npair * N],
                                 AF.Sigmoid, bias=B1COL)

            XW2ps = psXW2.tile([N, WAVE * H], f32, tag="xw2")
            for p in range(npair):
                # one full-array matmul computes BOTH graphs of the pair:
                # lhsT = [h1_a^T ; h1_b^T] stacked in partition halves,
                # rhs = block-diag[[W2,0],[0,W2]] -> out = [XW2_a | XW2_b]
                nc.tensor.matmul(XW2ps[:, (2 * p) * H:(2 * p + 2) * H],
                                 H1t[:, p * N:(p + 1) * N], W2BD,
                                 start=True, stop=True)
            XW2s = xw2p.tile([N, WAVE * H], f16)
            nc.vector.tensor_copy(XW2s[:, 0:gn * H], XW2ps[:, 0:gn * H])

            P2ps = psP2.tile([N, 4 * N], f32, tag="p2")
            for p in range(npair):
                ga, gb = g0 + 2 * p, g0 + 2 * p + 1
                nc.tensor.matmul(P2ps[0:64, p * N:(p + 1) * N],
                                 XW2s[:, (2 * p) * H:(2 * p + 1) * H],
                                 ATT[:, ga * N:(ga + 1) * N],
                                 start=True, stop=True)
                nc.tensor.matmul(P2ps[64:128, p * N:(p + 1) * N],
                                 XW2s[:, (2 * p + 1) * H:(2 * p + 2) * H],
                                 ATT[:, gb * N:(gb + 1) * N],
                                 start=True, stop=True)
            H2t = h2p.tile([N, 4 * N], f16)
            nc.scalar.activation(H2t[:, 0:npair * N], P2ps[:, 0:npair * N],
                                 AF.Sigmoid, bias=B2COL)

            for p in range(npair):
                # rhs = [[w3s, 0], [0, w3s]] -> out cols = [XW3_a, XW3_b]
                nc.tensor.matmul(Z3[:, g0 + 2 * p:g0 + 2 * p + 2],
                                 H2t[:, p * N:(p + 1) * N], W3BD,
                                 start=True, stop=True)


        # ---- final reduction: FPS[g] = sum_j SG[j,g] * Z3[j,g] ----
        MS = misc.tile([N, BC], f16, tag="ms")
        nc.vector.scalar_tensor_tensor(MS[:], Z3[:, 0:BC], 1.0,
                                       SG[:], ALU.mult, ALU.mult)
        # absorber: takes the MS (DVE) wait onto ACT's clock alone, so the
        # OUTS copy below carries only its PE wait.
        nc.scalar.copy(AW[0:1, 1:2], MS[0:1, 0:1])
        nc.tensor.matmul(Z3[0:1, 256:512], ONESCOL, MS[:],
                         start=True, stop=True)
        nc.scalar.activation(OUTS[:], Z3[0:1, 256:512], AF.Copy)
        # issue from the ACT queue: ordering after the OUTS copies is ACT
        # program order, so the descriptor needs no extra sem wait (the
        # interpreter's race detector flags this read as unsynchronized, but
        # the HWDGE descriptor is only generated once the copies retire).
        nc.scalar.dma_start(o_d[:], OUTS[:])

    _strip_same_engine_waits(nc)
    return nc


_ENGINE_SEM_PREFIX = {
    "Activation": "Activation",
    "DVE": "DVE",
    "PE": "PE",
    "Pool": "Pool",
    "SP": "SP",
}


def _strip_same_engine_waits(nc):
    """Drop sem waits where an instruction waits on its own engine's
    completion counter: engines retire in order, so such waits are always
    already satisfied at dispatch (the schedule would deadlock otherwise),
    and the TPB instruction structs only have room for one sync wait."""
    last_dma_sems = set()
    for fn in nc.m.functions:
        for blk in fn.blocks:
            for ins in blk.instructions:
                if type(ins).__name__ == "InstDMACopy":
                    si = ins.sync_info
                    if si and si.on_update:
                        last_dma_sems = {u.ant_name for u in si.on_update}
    for fn in nc.m.functions:
        for blk in fn.blocks:
            for ins in blk.instructions:
                si = ins.sync_info
                if si is None:
                    continue
                w = si.on_wait
                if not w or len(w) < 2:
                    continue
                eng = str(ins.engine).split(".")[-1]
                pref = _ENGINE_SEM_PREFIX.get(eng)
                if pref is None:
                    continue
                kept = [x for x in w if not x.ant_name.startswith(pref + "_")]
                if type(ins).__name__ == "InstDrain" and len(kept) > 1:
                    kept = [x for x in kept if x.ant_name in last_dma_sems]
                if len(kept) != len(w):
                    si.on_wait = kept


def _prep_inputs(x, edge_index, W1, b1, W2, b2, W3, b3):
    import ml_dtypes
    f16 = np.float16
    f8 = ml_dtypes.float8_e4m3

    rows = edge_index[:, 0, :].astype(np.int64)          # [B, E] sources (j)
    cols = edge_index[:, 1, :].astype(np.int64)          # [B, E] targets (i)

    # host-side degree normalization (self-loops included): deg >= 1
    flatc = (np.arange(B, dtype=np.int64)[:, None] * N + cols).ravel()
    deg = np.bincount(flatc, minlength=B * N).reshape(B, N).astype(np.float64)
    dinv = 1.0 / np.sqrt(deg + 1.0)                      # [B, N] f64

    # dense AT[b, j, i] = Ahat[i, j] = sum over edges (j->i) of
    # dinv[j]*dinv[i], plus the self-loop diagonal dinv^2.
    wgt = (np.take_along_axis(dinv, rows, 1)
           * np.take_along_axis(dinv, cols, 1)).ravel()
    flat = ((np.arange(B, dtype=np.int64)[:, None] * N + rows) * N + cols).ravel()
    at = np.bincount(flat, weights=wgt, minlength=B * N * N).reshape(B, N, N)
    idx = np.arange(N)
    at[:, idx, idx] += dinv * dinv
    s = at.sum(axis=2)                                   # [B, N] col sums of A
    at16 = at.astype(f16)
    s16 = s.astype(f16)

    cb = np.zeros((N, 200), np.float32)
    cb[:, 0:64] = W1
    cb[0:64, 64:128] = W2
    cb[64:128, 128:192] = W2
    w3s = W3.sum(axis=1, dtype=np.float64).astype(np.float32)
    cb[0:64, 192] = w3s
    cb[64:128, 193] = w3s
    cb[:, 194] = 1.0
    cb = cb.astype(f16)
    fcon = np.zeros((N, 4), np.float32)
    fcon[0:64, 0] = b1
    fcon[64:128, 0] = b1
    fcon[0:64, 1] = b2
    fcon[64:128, 1] = b2

    in_maps = []
    for c in range(NCORES):
        sl = slice(c * BC, (c + 1) * BC)
        xrt = np.ascontiguousarray(
            x[sl].transpose(2, 0, 1).reshape(N, BC * N)).astype(f8)
        atc = np.ascontiguousarray(
            at16[sl].transpose(1, 0, 2).reshape(N, BC * N))
        sgc = np.ascontiguousarray(s16[sl].T)            # [N, BC]
        in_maps.append(dict(xrt=xrt, at=atc, sg=sgc, cb=cb, fcon=fcon))
    return in_maps


def kernel(x, edge_index, W1, b1, W2, b2, W3, b3, _trace=False, _bench=0):
    from concourse.bass_utils import run_bass_kernel_spmd

    x = np.asarray(x, np.float32)
    edge_index = np.asarray(edge_index)
    b3 = np.asarray(b3, np.float32)
    in_maps = _prep_inputs(x, edge_index,
                           np.asarray(W1, np.float32), np.asarray(b1, np.float32),
                           np.asarray(W2, np.float32), np.asarray(b2, np.float32),
                           np.asarray(W3, np.float32), b3)
    if "nc" not in _CACHE:
        _CACHE["nc"] = _build_bass()
    nc = _CACHE["nc"]
    res = run_bass_kernel_spmd(nc, in_maps, list(range(NCORES)), trace=_trace)
    vals = np.concatenate([res.results[c]["o"][0] for c in range(NCORES)])
    _CACHE["last_result"] = res
    off = np.float32(b3.sum(dtype=np.float64) / H)
    return (vals / np.float32(N * H) + off).astype(np.float32)
